# revision 2
# baseline (speedup 1.0000x reference)
# Trainium2 Bass kernel v2 for nn_MultiHeadAttention_85933705658435
#
# Sharding (8 cores): batch (2-way) x head-group (4-way, 4 heads/core, as
# 2 pairs). Redundant LayerNorm per batch (cheaper than a collective).
#
# v2 changes vs baseline (319.6us):
#   - no DRAM bounces: ln transposed on the PE (identity matmul), V computed
#     token-major directly (stationary=lnT, moving=w_v)
#   - fp8e4 DoubleRow matmuls (0.5 cyc/row) for QKV, V, ctx, out-proj;
#     scores stay bf16 (row-tiled pairs)
#   - exp split across ACT (fp8 out, bias -ln4) and DVE/Pool via single-pass
#     Schraudolph (f32->int32 mult-add; bf16 upper-half bitcast view feeds a
#     bf16 ctx matmul)
#   - out-proj DoubleRow over the pr pair, overlapped into attention; out
#     written bf16 and summed f32 on the host
import numpy as np
import ml_dtypes

S, B, E = 2048, 2, 1024
H, D = 16, 64
HPC = 4              # heads per core
NCORES = 8
EPS = 1e-6
FQK = HPC * D        # 256
P = 128
TC = S // P          # 16 token chunks
ECH = E // P         # 8 e-chunks
QTS = 512            # q-tile size in attention
NQT = S // QTS       # 4

WQ_SCALE = 1.0
WK_SCALE = 1.0
WV_SCALE = 1.0
WO_SCALE = 1.0
SCORE_DESCALE = 1.0 / (WQ_SCALE * WK_SCALE)   # exp input scale
EXP_BIAS = -float(np.log(4.0))                # fp8 headroom; cancels in softmax
LOG2E = float(np.log2(np.e))
SCHRAU_A = (1 << 23) * LOG2E * SCORE_DESCALE
# Schraudolph bias: minimax const 366393, +2^15 compensates bf16 truncation,
# -2*2^23 applies EXP_BIAS (= exp/4) exactly.
SCHRAU_B = float((127 << 23) - 366393 + (1 << 15) - (2 << 23))

# exp engine per (kcp, head-in-pair): 'A' = ACT (fp8 ctx), 'D' = DVE,
# 'P' = Pool (both bf16-Schraudolph ctx). Head A slices stay on ACT so
# Vp_bf only needs odd local heads.
EXP_ENG = [
    ('A', 'D'), ('A', 'P'), ('A', 'D'), ('A', 'D'),
    ('A', 'P'), ('A', 'D'), ('A', 'P'), ('A', 'D'),
]

BF16 = ml_dtypes.bfloat16
F8 = ml_dtypes.float8_e4m3

_CACHE = {}


def _build_nc():
    from contextlib import ExitStack

    import concourse.bass as bass
    import concourse.tile as tile
    from concourse import bacc, mybir
    from concourse.tile import add_dep_helper
    from concourse.masks import make_identity

    dt = mybir.dt
    Alu = mybir.AluOpType
    Act = mybir.ActivationFunctionType
    DR = mybir.MatmulPerfMode.DoubleRow

    nc = bacc.Bacc(trn_type="TRN2")
    x_d = nc.dram_tensor("x", (S, E), dt.float32, kind="ExternalInput").ap()
    wqkv_d = nc.dram_tensor(
        "wqkv", (E, 3 * FQK), dt.bfloat16, kind="ExternalInput"
    ).ap()
    wo_d = nc.dram_tensor("wo", (FQK, E), dt.bfloat16, kind="ExternalInput").ap()
    out_d = nc.dram_tensor("out", (S, E), dt.bfloat16, kind="ExternalOutput").ap()

    with tile.TileContext(nc) as tc, ExitStack() as ctx:
        singles = ctx.enter_context(tc.tile_pool(name="singles", bufs=1))
        xp = ctx.enter_context(tc.tile_pool(name="xp", bufs=6))
        lnp = ctx.enter_context(tc.tile_pool(name="lnp", bufs=3))
        small = ctx.enter_context(tc.tile_pool(name="small", bufs=8))
        expp = ctx.enter_context(tc.tile_pool(name="expp", bufs=3))
        evac = ctx.enter_context(tc.tile_pool(name="evac", bufs=2))
        obp = ctx.enter_context(tc.tile_pool(name="obp", bufs=3))
        dram = ctx.enter_context(tc.tile_pool(name="dram", bufs=1, space="DRAM"))

        # persistent SBUF tensors
        lnT = singles.tile([P, ECH, S], dt.bfloat16)        # ln^T, e-chunked
        qkT = singles.tile([P, 4, S], dt.bfloat16)          # fc 0,1: Q^T; 2,3: K^T
        Vp = singles.tile([P, TC, HPC * (D + 1)], dt.float8e4)  # 4*V + ones
        Vb = singles.tile([P, TC, 2 * (D + 1)], dt.bfloat16)    # odd heads, bf16
        w_sb = singles.tile([P, ECH, 3 * FQK], dt.bfloat16)
        wo_sb = singles.tile([P, 2, E], dt.bfloat16)
        ident = singles.tile([P, P], dt.float8e4)
        eps_sb = singles.tile([P, 1], dt.float32)
        ebias = singles.tile([P, 1], dt.float32)
        warm = singles.tile([P, 512], dt.bfloat16)
        ctxn = singles.tile([P, 2, S], dt.bfloat16)         # normalized ctx^T / pair
        rc_dram = dram.tile([16, QTS], dt.float32)

        nc.vector.memset(eps_sb[:], EPS)
        nc.vector.memset(ebias[:], EXP_BIAS)
        nc.vector.memset(warm[:], 0.5)
        make_identity(nc, ident[:])

        # ---- Phase 1: LN -> lnb fp8 -> PE transpose -> lnT; QKV/V matmuls ----
        with tc.tile_pool(name="psP", bufs=1, space="PSUM") as psP:
            # warm the HAM clock gate while the first x DMAs land
            wps = psP.tile([P, 512], dt.float32, tag="warm", name="wps")
            for _ in range(72):
                nc.tensor.matmul(
                    wps[:], warm[:, 0:P], warm[:], start=True, stop=True
                )

            xbt = []
            for tp2 in range(TC // 2):  # two t-chunks per DMA
                xb = xp.tile([P, 2, E], dt.float32, tag="xb")
                q = nc.scalar if tp2 % 2 == 0 else nc.sync
                q.dma_start(
                    xb[:],
                    x_d[tp2 * 2 * P : (tp2 + 1) * 2 * P, :].rearrange(
                        "(c p) e -> p c e", p=P
                    ),
                )
                xbt.append(xb)
            # weights are first needed by the tg-0 QKV matmuls (~40us in);
            # keep them behind the x loads in the queues
            nc.sync.dma_start(w_sb[:], wqkv_d.rearrange("(c p) f -> p c f", p=P))
            nc.scalar.dma_start(wo_sb[:], wo_d.rearrange("(c p) e -> p c e", p=P))
            # ones at column D (denominator row), zeros at the pad column
            nc.vector.memset(
                Vb.rearrange("p t (h z) -> p t h z", z=D + 4)[:, :, :, D : D + 4],
                0.0,
            )
            nc.vector.memset(
                Vb.rearrange("p t (h z) -> p t h z", z=D + 4)[:, :, :, D : D + 1],
                1.0,
            )

            for tg in range(4):           # groups of 4 t-chunks (512 tokens)
                for ti in range(4):
                    t = tg * 4 + ti
                    xb = xbt[t // 2][:, t % 2, :]
                    st = small.tile([P, 2, 6], dt.float32, tag="st")
                    nc.vector.bn_stats(st[:, 0, :], xb[:, 0:512])
                    nc.vector.bn_stats(st[:, 1, :], xb[:, 512:1024])
                    mv = small.tile([P, 2], dt.float32, tag="mv")
                    nc.vector.bn_aggr(mv[:], st[:])
                    sd = small.tile([P, 1], dt.float32, tag="sd")
                    nc.scalar.activation(sd[:], mv[:, 1:2], Act.Sqrt, bias=eps_sb[:])
                    rs = small.tile([P, 1], dt.float32, tag="rs")
                    nc.vector.reciprocal(rs[:], sd[:])
                    lnb = lnp.tile([P, E], dt.bfloat16, tag="lnb")
                    # ln = x*rs + (-mu*rs): ACT for even chunks, Pool+DVE odd
                    nb = small.tile([P, 1], dt.float32, tag="nb")
                    nc.vector.tensor_scalar(
                        nb[:], mv[:, 0:1], rs[:], -1.0, Alu.mult, Alu.mult
                    )
                    if t % 2 == 0:
                        nc.scalar.activation(
                            lnb[:], xb[:], Act.Identity, bias=nb[:], scale=rs[:]
                        )
                    else:
                        nc.gpsimd.tensor_scalar(
                            lnb[:], xb[:], rs[:], nb[:], Alu.mult, Alu.add
                        )
                    # PE transpose: 8 tiles of [128,128] -> lnT[:, :, t*128...]
                    tps = psP.tile([P, ECH, P], dt.float8e4, tag="tp", name="tps",
                                   bufs=2)
                    for c in range(ECH):
                        nc.tensor.transpose(
                            tps[:, c, :], lnb[:, c * P : (c + 1) * P], ident[:]
                        )
                    dst = lnT[:, :, t * P : (t + 1) * P]
                    if t % 3 == 0:
                        nc.vector.tensor_copy(dst, tps[:])
                    elif t % 3 == 1:
                        nc.scalar.activation(dst, tps[:], Act.Identity)
                    else:
                        nc.gpsimd.tensor_copy(dst, tps[:])
                    # V token-major: psum [128tok, 256] = lnT_t^T @ w_v
                    vps = psP.tile([P, FQK], dt.float32, tag="v", name="vps",
                                   bufs=2)
                    for ec in range(ECH):
                        nc.tensor.matmul(
                            vps[:],
                            lnT[:, ec, t * P : (t + 1) * P],
                            w_sb[:, ec, 2 * FQK : 3 * FQK],
                            start=(ec == 0),
                            stop=(ec == ECH - 1),
                        )
                    vh = vps.rearrange("p (h z) -> p h z", z=D)
                    nc.vector.tensor_copy(
                        Vp[:, t, :].rearrange("p (h z) -> p h z", z=D + 4)[
                            :, :, 0:D
                        ],
                        vh[:, :, :],
                    )
                    nc.gpsimd.tensor_copy(
                        Vb[:, t, :].rearrange("p (h z) -> p h z", z=D + 4)[
                            :, :, 0:D
                        ],
                        bass.AP(
                            tensor=vh.tensor,
                            offset=vh.offset + D,
                            ap=[list(vh.ap[0]), [2 * D, 2], [1, D]],
                        ),
                    )
                # Q^T,K^T for this 512-token group (feature-major)
                t0 = tg * 512
                for fc in range(4):
                    qp = psP.tile([P, 512], dt.float32, tag="qk", name=f"qp{fc % 2}",
                                  bufs=3)
                    for ec in range(ECH):
                        nc.tensor.matmul(
                            qp[:],
                            w_sb[:, ec, fc * P : (fc + 1) * P],
                            lnT[:, ec, t0 : t0 + 512],
                            start=(ec == 0),
                            stop=(ec == ECH - 1),
                        )
                    dstq = qkT[:, fc, t0 : t0 + 512]
                    if fc % 3 == 0:
                        nc.vector.tensor_copy(dstq, qp[:])
                    elif fc % 3 == 1:
                        nc.scalar.activation(dstq, qp[:], Act.Identity)
                    else:
                        nc.gpsimd.tensor_copy(dstq, qp[:])

        # ---- Phase 2: attention + overlapped out-proj ----
        with tc.tile_pool(name="psB", bufs=1, space="PSUM") as psB:
            for qt in range(NQT):
                q0 = qt * QTS
                for pr in range(2):
                    cps = [
                        psB.tile([D + 4, QTS], dt.float32, tag=f"ctx{h}",
                                 name=f"cps{h}")
                        for h in range(2)
                    ]
                    stiles = {}
                    etiles = {}

                    def scores(kc, h):
                        sT = psB.tile([P, QTS], dt.float32, tag=f"s{(2 * kc + h) % 3}",
                                      name="sT")
                        nc.tensor.matmul(
                            sT[:],
                            qkT[64 * h : 64 * h + 64, 2 + pr, kc * P : (kc + 1) * P],
                            qkT[64 * h : 64 * h + 64, pr, q0 : q0 + QTS],
                            start=True,
                            stop=True,
                            tile_position=(64 * h, 0),
                        )
                        stiles[(kc, h)] = sT

                    def exps(kc, h):
                        kcp, j = kc // 2, kc % 2
                        eng = EXP_ENG[kcp][h]
                        sT = stiles.pop((kc, h))
                        if eng == 'A':
                            if j == 0:
                                etiles[(kcp, h)] = expp.tile(
                                    [P, 2, QTS], dt.float8e4, tag=f"e8{h}", name="e8"
                                )
                            nc.scalar.activation(
                                etiles[(kcp, h)][:, j, :], sT[:], Act.Exp,
                                bias=ebias[:], scale=SCORE_DESCALE,
                            )
                        else:
                            if j == 0:
                                etiles[(kcp, h)] = expp.tile(
                                    [P, 2, QTS], dt.int32, tag=f"ei{h}", name="ei"
                                )
                            e = nc.vector if eng == 'D' else nc.gpsimd
                            e.tensor_scalar(
                                etiles[(kcp, h)][:, j, :], sT[:],
                                SCHRAU_A, SCHRAU_B, Alu.mult, Alu.add,
                            )

                    def ctx_mms(kcp, h):
                        hh = pr * 2 + h
                        et = etiles.pop((kcp, h))
                        if EXP_ENG[kcp][h] == 'A':
                            nc.tensor.matmul(
                                cps[h][:],
                                Vp[:, 2 * kcp : 2 * kcp + 2,
                                   hh * (D + 4) : (hh + 1) * (D + 4)],
                                et[:],
                                start=(kcp == 0),
                                stop=(kcp == TC // 2 - 1),
                                perf_mode=DR,
                                skip_group_check=True,
                            )
                        else:
                            ebf = et[:].bitcast(dt.bfloat16)
                            for j in range(2):
                                mv_ap = bass.AP(
                                    tensor=ebf.tensor,
                                    offset=ebf.offset + 2 * j * QTS + 1,
                                    ap=[list(ebf.ap[0]), [2, QTS]],
                                )
                                nc.tensor.matmul(
                                    cps[h][:],
                                    Vb[:, 2 * kcp + j,
                                       pr * (D + 1) : (pr + 1) * (D + 1)],
                                    mv_ap,
                                    start=(kcp == 0 and j == 0),
                                    stop=(kcp == TC // 2 - 1 and j == 1),
                                    skip_group_check=True,
                                )

                    # software pipeline: ctx(kcp) emitted between scores of
                    # the next chunk so the PE stays busy during exp
                    for kc in range(TC):
                        for h in range(2):
                            scores(kc, h)
                            exps(kc, h)
                        if kc % 2 == 1 and kc >= 3:
                            ctx_mms(kc // 2 - 1, 0)
                            ctx_mms(kc // 2 - 1, 1)
                    ctx_mms(TC // 2 - 1, 0)
                    ctx_mms(TC // 2 - 1, 1)

                    # evacuate ctx psums, normalize by the ones-row denominator
                    for h in range(2):
                        ctxu = evac.tile([D + 1, QTS], dt.float32, tag=f"cu{h}",
                                         name=f"ctxu{h}")
                        nc.gpsimd.tensor_copy(ctxu[:], cps[h][:])
                        dnp = small.tile([P, QTS // P], dt.float32, tag="dnp")
                        nc.sync.dma_start(dnp[:], ctxu[D : D + 1, :])
                        rcp = small.tile([P, QTS // P], dt.float32, tag="rcp")
                        # denominator*W V_SCALE; reciprocal folds the V descale
                        nc.vector.tensor_scalar_mul(dnp[:], dnp[:], WV_SCALE)
                        nc.vector.reciprocal(rcp[:], dnp[:])
                        slot = (qt * 2 + pr) * 2 + h
                        rc_row = rc_dram[slot : slot + 1, :]
                        wr = nc.sync.dma_start(rc_row, rcp[:])
                        bcs = evac.tile([64, QTS], dt.float32, tag="bcs")
                        rc_bcast = bass.AP(
                            tensor=rc_row.tensor,
                            offset=rc_row.offset,
                            ap=[[0, 64]] + list(rc_row.ap[1:]),
                        )
                        rd = nc.sync.dma_start(bcs[:], rc_bcast)
                        add_dep_helper(rd.ins, wr.ins, True, "recip RAW via dram")
                        if h == 0:
                            nc.vector.tensor_tensor(
                                ctxn[0:64, pr, q0 : q0 + QTS],
                                ctxu[0:D, :],
                                bcs[:],
                                Alu.mult,
                            )
                        else:
                            tmpn = evac.tile([64, QTS], dt.bfloat16, tag="tmpn")
                            nc.gpsimd.tensor_tensor(
                                tmpn[:], ctxu[0:D, :], bcs[:], Alu.mult
                            )
                            nc.sync.dma_start(
                                ctxn[64:128, pr, q0 : q0 + QTS], tmpn[:]
                            )

                # out-proj for this q-tile
                if qt == NQT - 1:
                    # hold the clock while the final normalize drains
                    wfl = psB.tile([P, 2, QTS], dt.float32, tag="s1", name="sT")
                    for _ in range(28):
                        nc.tensor.matmul(
                            wfl[:, 0, :], warm[:, 0:P], warm[:],
                            start=True, stop=True,
                        )
                for ti in range(QTS // P):
                    t0 = q0 + ti * P
                    po = psB.tile([P, E], dt.float32, tag="po", name="po")
                    for et in range(2):
                        for pr in range(2):
                            nc.tensor.matmul(
                                po[:, et * 512 : (et + 1) * 512],
                                ctxn[:, pr, t0 : t0 + P],
                                wo_sb[:, pr, et * 512 : (et + 1) * 512],
                                start=(pr == 0),
                                stop=(pr == 1),
                            )
                    ob = obp.tile([P, E], dt.bfloat16, tag="ob")
                    if ti % 2 == 0:
                        nc.vector.tensor_scalar_mul(ob[:], po[:], 1.0 / WO_SCALE)
                    else:
                        nc.scalar.activation(
                            ob[:], po[:], Act.Identity, scale=1.0 / WO_SCALE
                        )
                    q = nc.scalar if ti % 2 == 0 else nc.sync
                    q.dma_start(out_d[t0 : t0 + P, :], ob[:])
            # hold the HAM clock through the final evac/normalize/DMA drain
            wpt = psB.tile([P, 512], dt.float32, tag="po0", name="wpt")
            for _ in range(24):
                nc.tensor.matmul(
                    wpt[:], warm[:, 0:P], warm[:], start=True, stop=True
                )

    nc.compile()
    return nc


def make_in_maps(x, ln_scale, w_qkv, w_out):
    w = np.asarray(w_qkv, np.float32) * np.asarray(ln_scale, np.float32)[:, None]
    wo = np.asarray(w_out, np.float32)
    in_maps = []
    for c in range(NCORES):
        b, g = divmod(c, 4)
        h0 = g * HPC
        wq = w[:, h0 * D : (h0 + HPC) * D]
        wk = w[:, H * D + h0 * D : H * D + (h0 + HPC) * D]
        wv = w[:, 2 * H * D + h0 * D : 2 * H * D + (h0 + HPC) * D]
        in_maps.append(
            {
                "x": np.ascontiguousarray(np.asarray(x, np.float32)[:, b, :]),
                "wqkv": np.ascontiguousarray(
                    np.concatenate([wq, wk, wv], axis=1)
                ).astype(BF16),
                "wo": np.ascontiguousarray(
                    wo[h0 * D : (h0 + HPC) * D, :]
                ).astype(BF16),
            }
        )
    return in_maps


def get_nc():
    if "nc" not in _CACHE:
        _CACHE["nc"] = _build_nc()
    return _CACHE["nc"]


def assemble(results):
    out = np.empty((S, B, E), np.float32)
    for b in range(B):
        acc = results[4 * b]["out"].astype(np.float32)
        for g in range(1, 4):
            acc = acc + results[4 * b + g]["out"].astype(np.float32)
        out[:, b, :] = acc
    return out


def kernel(x, ln_scale, w_qkv, w_out):
    from concourse.bass_utils import run_bass_kernel_spmd

    nc = get_nc()
    in_maps = make_in_maps(x, ln_scale, w_qkv, w_out)
    res = run_bass_kernel_spmd(nc, in_maps, core_ids=list(range(NCORES)))
    return assemble(res.results)
